# revision 1
# baseline (speedup 1.0000x reference)
"""Mutual channel attention (sparse_attention) TRN2 Bass kernel.

Problem: x1, x2 of shape (16, 512, 64, 64) fp32.
  q = x1.reshape(B, C, D), k = x2.reshape(B, C, D), D = 4096, scale = 1/64
  S   = q @ k^T * scale                       [B, 512, 512]
  outA = softmax_rows(S) @ k                  -> (16, 512, 64, 64)
  outB = softmax_rows(S^T) @ q                -> (16, 512, 64, 64)

Key algebra: without max-subtraction (scores ~ N(0,1), safe in fp32),
P = exp(S*scale) serves BOTH directions; only the normalization sums
differ (row sums of P for A, column sums of P for B).
  outA[c,:] = (P @ k)[c,:]   / rowsum_P[c]
  outB[e,:] = (P^T @ q)[e,:] / colsum_P[e]

Sharding: pure data parallel, 2 batches per core across 8 cores.

All matmuls run in float32r (single-pass fp32, 1 cycle/row at N=512,
~2e-4 rel err vs the fp32 reference on HW). q/k live in SBUF as 4x4
quarter tiles [128, 1024] so slots free progressively during the
d-outer out phase and the next batch's loads overlap compute.

Per-core per-batch schedule:
  1. Load q,k quarter tiles (quarter-major: the scores phase can start
     after the first 4.2MB lands).
  2. Scores: per 128-wide d-chunk, PE-transpose 4 q-blocks + 4
     k-blocks into [128,512] psum staging, copy to SBUF (q-half on
     DVE, k-half on ACT), 4 accumulating matmuls into resident S banks.
  3. exp via ScalarE with fused *1/64 scale and fused row-sum.
  4. PE-transpose P -> P_ec with fused column-sum on the copy-out.
  5. out_a = P_ec.T @ k (d-outer, frees k quarters early for the next
     batch's k loads), then out_b = P_ce.T @ q (same for q);
     normalization folded into the PSUM->SBUF copy as a per-partition
     scale; copies alternate DVE/ACT.
"""

import numpy as np

B, C, D = 16, 512, 4096
N_CORES = 8
B_PER_CORE = B // N_CORES  # 2
CC = C // 128  # 4 c-chunks
DC = D // 128  # 32 d-chunks
NQ = 8  # d-slices per row-chunk tile ([128,512] eighths: halves first-load wait, finer frees)
QW = D // NQ  # 1024 quarter width
NG = D // 512  # 8 d-groups of 512 in the out phase

_COMPILED = {}


def _build():
    import concourse.mybir as mybir
    from concourse import bacc, tile

    f32 = mybir.dt.float32
    f32r = mybir.dt.float32r
    bf16 = mybir.dt.bfloat16
    AF = mybir.ActivationFunctionType
    ROWS = B_PER_CORE * C  # 1024

    nc = bacc.Bacc(None, target_bir_lowering=False)
    x1 = nc.declare_dram_parameter("x1", [ROWS, D], f32r, isOutput=False)
    x2 = nc.declare_dram_parameter("x2", [ROWS, D], f32r, isOutput=False)
    ident = nc.declare_dram_parameter("ident", [128, 128], f32r, isOutput=False)
    outA = nc.declare_dram_parameter("outA", [ROWS, D], f32, isOutput=True)
    outB = nc.declare_dram_parameter("outB", [ROWS, D], f32, isOutput=True)

    with tile.TileContext(nc) as tc:
        with (
            tc.tile_pool(name="const", bufs=1) as constp,
            tc.tile_pool(name="qk", bufs=1) as qk,
            tc.tile_pool(name="stg_sb", bufs=3) as stgsb,
            tc.tile_pool(name="pp", bufs=2) as pp,
            tc.tile_pool(name="rp", bufs=2) as rp,
            tc.tile_pool(name="osb", bufs=6) as osb,
            tc.tile_pool(name="sps", bufs=1, space="PSUM") as sps,
            tc.tile_pool(name="stgps", bufs=4, space="PSUM") as stgps,
        ):
            idt = constp.tile([128, 128], f32r)
            nc.sync.dma_start(idt[:], ident[:])

            for b in range(B_PER_CORE):
                r0 = b * C
                # ---- load q, k as quarter tiles, quarter-major ----
                q = [[None] * NQ for _ in range(CC)]
                k = [[None] * NQ for _ in range(CC)]
                for h in range(NQ):
                    for cc in range(CC):
                        rows = slice(r0 + cc * 128, r0 + (cc + 1) * 128)
                        cols = slice(h * QW, (h + 1) * QW)
                        qt = qk.tile(
                            [128, QW], f32r, tag=f"q{cc}_{h}", name=f"q{cc}_{h}"
                        )
                        kt = qk.tile(
                            [128, QW], f32r, tag=f"k{cc}_{h}", name=f"k{cc}_{h}"
                        )
                        nc.sync.dma_start(qt[:], x1[rows, cols])
                        nc.sync.dma_start(kt[:], x2[rows, cols])
                        q[cc][h] = qt
                        k[cc][h] = kt

                # ---- scores: S_ce[cc] accumulates over 32 d-chunks ----
                s_ps = [
                    sps.tile([128, C], f32, tag=f"s{cc}", name=f"s{cc}")
                    for cc in range(CC)
                ]
                for dc in range(DC):
                    h, off = divmod(dc * 128, QW)
                    dsl = slice(off, off + 128)
                    qt_ps = stgps.tile([128, 512], f32r, tag="st", name="qt_ps")
                    kt_ps = stgps.tile([128, 512], f32r, tag="st", name="kt_ps")
                    for cc in range(CC):
                        csl = slice(cc * 128, (cc + 1) * 128)
                        nc.tensor.transpose(qt_ps[:, csl], q[cc][h][:, dsl], idt[:])
                        nc.tensor.transpose(kt_ps[:, csl], k[cc][h][:, dsl], idt[:])
                    qt_sb = stgsb.tile([128, 512], f32r, tag="qt_sb", name="qt_sb")
                    kt_sb = stgsb.tile([128, 512], f32r, tag="kt_sb", name="kt_sb")
                    nc.vector.tensor_copy(qt_sb[:], qt_ps[:])
                    nc.scalar.activation(kt_sb[:], kt_ps[:], AF.Copy)
                    for cc in range(CC):
                        nc.tensor.matmul(
                            s_ps[cc][:],
                            qt_sb[:, cc * 128 : (cc + 1) * 128],
                            kt_sb[:],
                            start=(dc == 0),
                            stop=(dc == DC - 1),
                        )

                # ---- exp + row sums (direction A) ----
                p_ce = []
                rinv_a = []
                for cc in range(CC):
                    p = pp.tile([128, C], f32r, tag=f"pce{cc}", name=f"pce{cc}")
                    rs = rp.tile([128, 1], f32, tag=f"rsa{cc}", name=f"rsa{cc}")
                    nc.scalar.activation(
                        p[:], s_ps[cc][:], AF.Exp, scale=1.0 / 64.0, accum_out=rs[:]
                    )
                    ri = rp.tile([128, 1], f32, tag=f"ria{cc}", name=f"ria{cc}")
                    nc.vector.reciprocal(ri[:], rs[:])
                    p_ce.append(p)
                    rinv_a.append(ri)

                # ---- transpose P -> P_ec + column sums (direction B) ----
                p_ec = []
                rinv_b = []
                for ec in range(CC):
                    esl = slice(ec * 128, (ec + 1) * 128)
                    t_ps = stgps.tile([128, 512], f32r, tag="st", name="pt_ps")
                    for cc in range(CC):
                        nc.tensor.transpose(
                            t_ps[:, cc * 128 : (cc + 1) * 128], p_ce[cc][:, esl], idt[:]
                        )
                    p = pp.tile([128, C], f32r, tag=f"pec{ec}", name=f"pec{ec}")
                    rs = rp.tile([128, 1], f32, tag=f"rsb{ec}", name=f"rsb{ec}")
                    nc.scalar.activation(p[:], t_ps[:], AF.Copy, accum_out=rs[:])
                    ri = rp.tile([128, 1], f32, tag=f"rib{ec}", name=f"rib{ec}")
                    nc.vector.reciprocal(ri[:], rs[:])
                    p_ec.append(p)
                    rinv_b.append(ri)

                # ---- out_a = (P_ec.T @ k) * rinv_a, d-outer frees k early ----
                for g in range(NG):
                    h, off = divmod(g * 512, QW)
                    dsl = slice(off, off + 512)
                    for cc in range(CC):
                        csl = slice(cc * 128, (cc + 1) * 128)
                        o_ps = stgps.tile([128, 512], f32, tag="st", name="oa_ps")
                        for ec in range(CC):
                            nc.tensor.matmul(
                                o_ps[:],
                                p_ec[ec][:, csl],
                                k[ec][h][:, dsl],
                                start=(ec == 0),
                                stop=(ec == CC - 1),
                            )
                        o_sb = osb.tile([128, 512], f32, tag="osb", name="oa_sb")
                        if cc % 2 == 0:
                            nc.vector.tensor_scalar_mul(o_sb[:], o_ps[:], rinv_a[cc][:])
                        else:
                            nc.scalar.activation(
                                o_sb[:], o_ps[:], AF.Copy, scale=rinv_a[cc][:]
                            )
                        nc.sync.dma_start(
                            outA[
                                r0 + cc * 128 : r0 + (cc + 1) * 128,
                                g * 512 : (g + 1) * 512,
                            ],
                            o_sb[:],
                        )

                # ---- out_b = (P_ce.T @ q) * rinv_b, d-outer frees q early ----
                for g in range(NG):
                    h, off = divmod(g * 512, QW)
                    dsl = slice(off, off + 512)
                    for ec in range(CC):
                        esl = slice(ec * 128, (ec + 1) * 128)
                        o_ps = stgps.tile([128, 512], f32, tag="st", name="ob_ps")
                        for cc in range(CC):
                            nc.tensor.matmul(
                                o_ps[:],
                                p_ce[cc][:, esl],
                                q[cc][h][:, dsl],
                                start=(cc == 0),
                                stop=(cc == CC - 1),
                            )
                        o_sb = osb.tile([128, 512], f32, tag="osb", name="ob_sb")
                        if ec % 2 == 0:
                            nc.vector.tensor_scalar_mul(o_sb[:], o_ps[:], rinv_b[ec][:])
                        else:
                            nc.scalar.activation(
                                o_sb[:], o_ps[:], AF.Copy, scale=rinv_b[ec][:]
                            )
                        nc.sync.dma_start(
                            outB[
                                r0 + ec * 128 : r0 + (ec + 1) * 128,
                                g * 512 : (g + 1) * 512,
                            ],
                            o_sb[:],
                        )

    nc.finalize()
    return nc


def _get_nc():
    if "nc" not in _COMPILED:
        _COMPILED["nc"] = _build()
    return _COMPILED["nc"]


def kernel(x1: np.ndarray, x2: np.ndarray):
    from concourse.bass_utils import run_bass_kernel_spmd

    nc = _get_nc()
    x1 = np.ascontiguousarray(x1, dtype=np.float32)
    x2 = np.ascontiguousarray(x2, dtype=np.float32)
    ident = np.eye(128, dtype=np.float32)

    in_maps = []
    for i in range(N_CORES):
        sl = slice(i * B_PER_CORE, (i + 1) * B_PER_CORE)
        in_maps.append(
            {
                "x1": x1[sl].reshape(B_PER_CORE * C, D),
                "x2": x2[sl].reshape(B_PER_CORE * C, D),
                "ident": ident,
            }
        )

    res = None
    for attempt in range(3):
        try:
            res = run_bass_kernel_spmd(nc, in_maps, list(range(N_CORES))).results
            break
        except Exception:
            if attempt == 2:
                raise
    assert res is not None

    outA = np.empty((B, C, 64, 64), dtype=np.float32)
    outB = np.empty((B, C, 64, 64), dtype=np.float32)
    for i in range(N_CORES):
        sl = slice(i * B_PER_CORE, (i + 1) * B_PER_CORE)
        outA[sl] = res[i]["outA"].reshape(B_PER_CORE, C, 64, 64)
        outB[sl] = res[i]["outB"].reshape(B_PER_CORE, C, 64, 64)
    return outA, outB



# revision 4
# speedup vs baseline: 1.2890x; 1.2890x over previous
"""Mutual channel attention (sparse_attention) TRN2 Bass kernel — v2 (bf16).

Problem: x1, x2 of shape (16, 512, 64, 64) fp32.
  q = x1.reshape(B, C, D), k = x2.reshape(B, C, D), D = 4096, scale = 1/64
  S   = q @ k^T * scale                       [B, 512, 512]
  outA = softmax_rows(S) @ k                  -> (16, 512, 64, 64)
  outB = softmax_rows(S^T) @ q                -> (16, 512, 64, 64)

Key algebra: without max-subtraction (scores ~ N(0,1), safe in fp32),
P = exp(S*scale) serves BOTH directions; only the normalization sums
differ (row sums of P for A, column sums of P for B).

v2 vs v1 (285 us baseline):
- All matmuls in bf16 (1 cyc/row, rel err ~3e-3 vs 2e-2 budget). The
  host casts inputs to bf16 and also provides q^T/k^T in a chunked
  layout, so the per-d-chunk PE transposes (40% extra PE work in v1)
  disappear entirely. Outputs return as bf16 and are upcast on host.
- PE work per batch is just the three 512x512x4096 GEMMs + 16 P-block
  transposes: ~199k cycles -> ~166 us/core for 2 batches at 2.4 GHz.
- DMA per core: 33.6 MB in + 16.8 MB out = 50.4 MB (~141 us at 358
  GB/s), fully overlapped with compute.

Sharding: pure data parallel, 2 batches per core across 8 cores.

Per-core per-batch schedule:
  1. qT/kT piece loads (sync ring), k/q natural loads (scalar ring).
  2. Scores: per 128-d-chunk, 4 accumulating matmuls into resident
     S psum banks (stationary = qT slice, moving = kT chunk).
  3. exp via ACT with fused *1/64 scale and fused row-sum (dir A).
  4. PE-transpose P -> P_ec with fused column-sum on the ACT copy-out
     (dir B); DVE reciprocals.
  5. out_a = P_ec.T @ k, cc-outer with [128,4096] bf16 staging per
     c-chunk, normalization folded into the PSUM->SBUF copy
     (alternating DVE/ACT), one 1 MB store per chunk. Then
     out_b = P_ce.T @ q symmetrically.
"""

import numpy as np

B, C, D = 16, 512, 4096
N_CORES = 8
B_PER_CORE = B // N_CORES  # 2
CC = C // 128  # 4 c-chunks
DC = D // 128  # 32 d-chunks
NP = 4  # qT/kT pieces per batch
PCH = DC // NP  # 8 d-chunks per piece
NG = D // 512  # 8 d-groups of 512 in the out phase

_COMPILED = {}


def _build():
    import concourse.mybir as mybir
    from concourse import bacc, tile

    f32 = mybir.dt.float32
    bf16 = mybir.dt.bfloat16
    AF = mybir.ActivationFunctionType
    ROWS = B_PER_CORE * C  # 1024

    nc = bacc.Bacc(None, target_bir_lowering=False)
    x1 = nc.declare_dram_parameter("x1", [ROWS, D], bf16, isOutput=False)
    x2 = nc.declare_dram_parameter("x2", [ROWS, D], bf16, isOutput=False)
    # Per-batch transposed copies, chunk-major: row (b*DC + dc)*128 + p
    # holds q[b, :, dc*128+p] (c along the 512 columns).
    x1t = nc.declare_dram_parameter("x1t", [B_PER_CORE * D, C], bf16, isOutput=False)
    x2t = nc.declare_dram_parameter("x2t", [B_PER_CORE * D, C], bf16, isOutput=False)
    ident = nc.declare_dram_parameter("ident", [128, 128], bf16, isOutput=False)
    outA = nc.declare_dram_parameter("outA", [ROWS, D], bf16, isOutput=True)
    outB = nc.declare_dram_parameter("outB", [ROWS, D], bf16, isOutput=True)

    with tile.TileContext(nc) as tc:
        with (
            tc.tile_pool(name="const", bufs=1) as constp,
            tc.tile_pool(name="tp", bufs=1) as tp,
            tc.tile_pool(name="nat", bufs=1) as nat,
            tc.tile_pool(name="pp", bufs=1) as pp,
            tc.tile_pool(name="rp", bufs=2) as rp,
            tc.tile_pool(name="ost", bufs=2) as ost,
            tc.tile_pool(name="sps", bufs=1, space="PSUM") as sps,
            tc.tile_pool(name="ops", bufs=4, space="PSUM") as ops,
        ):
            idt = constp.tile([128, 128], bf16)
            nc.sync.dma_start(idt[:], ident[:])

            for b in range(B_PER_CORE):
                r0 = b * C
                # ---- transposed piece loads (sync ring), q/k interleaved ----
                qT = []
                kT = []
                for a in range(NP):
                    rows = slice(b * D + a * PCH * 128, b * D + (a + 1) * PCH * 128)
                    qt = tp.tile([128, PCH, C], bf16, tag=f"qT{a}", name=f"qT{a}")
                    nc.sync.dma_start(
                        qt[:], x1t[rows, :].rearrange("(di p) c -> p di c", p=128)
                    )
                    kt = tp.tile([128, PCH, C], bf16, tag=f"kT{a}", name=f"kT{a}")
                    nc.sync.dma_start(
                        kt[:], x2t[rows, :].rearrange("(di p) c -> p di c", p=128)
                    )
                    qT.append(qt)
                    kT.append(kt)

                # ---- natural loads (scalar ring): k first (out_a), then q ----
                k = []
                q = []
                for cc in range(CC):
                    rows = slice(r0 + cc * 128, r0 + (cc + 1) * 128)
                    ktile = nat.tile([128, D], bf16, tag=f"k{cc}", name=f"k{cc}")
                    nc.scalar.dma_start(ktile[:], x2[rows, :])
                    k.append(ktile)
                for cc in range(CC):
                    rows = slice(r0 + cc * 128, r0 + (cc + 1) * 128)
                    qtile = nat.tile([128, D], bf16, tag=f"q{cc}", name=f"q{cc}")
                    nc.scalar.dma_start(qtile[:], x1[rows, :])
                    q.append(qtile)

                # ---- scores: S_ce[cc] accumulates over 32 d-chunks ----
                s_ps = [
                    sps.tile([128, C], f32, tag=f"s{cc}", name=f"s{cc}")
                    for cc in range(CC)
                ]
                for dc in range(DC):
                    a, di = divmod(dc, PCH)
                    for cc in range(CC):
                        nc.tensor.matmul(
                            s_ps[cc][:],
                            qT[a][:, di, cc * 128 : (cc + 1) * 128],
                            kT[a][:, di, :],
                            start=(dc == 0),
                            stop=(dc == DC - 1),
                        )

                # ---- exp + row sums (direction A) ----
                p_ce = []
                rinv_a = []
                for cc in range(CC):
                    p = pp.tile([128, C], bf16, tag=f"pce{cc}", name=f"pce{cc}")
                    rs = rp.tile([128, 1], f32, tag=f"rsa{cc}", name=f"rsa{cc}")
                    nc.scalar.activation(
                        p[:], s_ps[cc][:], AF.Exp, scale=1.0 / 64.0, accum_out=rs[:]
                    )
                    ri = rp.tile([128, 1], f32, tag=f"ria{cc}", name=f"ria{cc}")
                    nc.vector.reciprocal(ri[:], rs[:])
                    p_ce.append(p)
                    rinv_a.append(ri)

                # ---- transpose P -> P_ec + column sums (direction B) ----
                stg = [
                    ops.tile([128, C], bf16, tag="st", name=f"pt{ec}")
                    for ec in range(CC)
                ]
                for cc in range(CC):
                    for ec in range(CC):
                        nc.tensor.transpose(
                            stg[ec][:, cc * 128 : (cc + 1) * 128],
                            p_ce[cc][:, ec * 128 : (ec + 1) * 128],
                            idt[:],
                        )
                p_ec = []
                rinv_b = []
                for ec in range(CC):
                    p = pp.tile([128, C], bf16, tag=f"pec{ec}", name=f"pec{ec}")
                    rs = rp.tile([128, 1], f32, tag=f"rsb{ec}", name=f"rsb{ec}")
                    nc.scalar.activation(p[:], stg[ec][:], AF.Copy, accum_out=rs[:])
                    ri = rp.tile([128, 1], f32, tag=f"rib{ec}", name=f"rib{ec}")
                    nc.vector.reciprocal(ri[:], rs[:])
                    p_ec.append(p)
                    rinv_b.append(ri)

                # ---- out_a = (P_ec.T @ k) * rinv_a, cc-outer, wide staging ----
                for cc in range(CC):
                    csl = slice(cc * 128, (cc + 1) * 128)
                    oa = ost.tile([128, D], bf16, tag="ost", name=f"oa{cc}")
                    for g in range(NG):
                        dsl = slice(g * 512, (g + 1) * 512)
                        o_ps = ops.tile([128, 512], f32, tag="st", name="oa_ps")
                        for ec in range(CC):
                            nc.tensor.matmul(
                                o_ps[:],
                                p_ec[ec][:, csl],
                                k[ec][:, dsl],
                                start=(ec == 0),
                                stop=(ec == CC - 1),
                            )
                        if g % 2 == 0:
                            nc.vector.tensor_scalar_mul(
                                oa[:, dsl], o_ps[:], rinv_a[cc][:]
                            )
                        else:
                            nc.scalar.activation(
                                oa[:, dsl], o_ps[:], AF.Copy, scale=rinv_a[cc][:]
                            )
                    nc.scalar.dma_start(
                        outA[r0 + cc * 128 : r0 + (cc + 1) * 128, :], oa[:]
                    )

                # ---- out_b = (P_ce.T @ q) * rinv_b, ec-outer, wide staging ----
                for ec in range(CC):
                    esl = slice(ec * 128, (ec + 1) * 128)
                    ob = ost.tile([128, D], bf16, tag="ost", name=f"ob{ec}")
                    for g in range(NG):
                        dsl = slice(g * 512, (g + 1) * 512)
                        o_ps = ops.tile([128, 512], f32, tag="st", name="ob_ps")
                        for cc in range(CC):
                            nc.tensor.matmul(
                                o_ps[:],
                                p_ce[cc][:, esl],
                                q[cc][:, dsl],
                                start=(cc == 0),
                                stop=(cc == CC - 1),
                            )
                        if g % 2 == 0:
                            nc.vector.tensor_scalar_mul(
                                ob[:, dsl], o_ps[:], rinv_b[ec][:]
                            )
                        else:
                            nc.scalar.activation(
                                ob[:, dsl], o_ps[:], AF.Copy, scale=rinv_b[ec][:]
                            )
                    nc.scalar.dma_start(
                        outB[r0 + ec * 128 : r0 + (ec + 1) * 128, :], ob[:]
                    )

    nc.finalize()
    return nc


def _get_nc():
    if "nc" not in _COMPILED:
        _COMPILED["nc"] = _build()
    return _COMPILED["nc"]


def make_in_maps(x1: np.ndarray, x2: np.ndarray):
    import ml_dtypes

    bf = ml_dtypes.bfloat16
    x1 = np.asarray(x1, dtype=np.float32).reshape(B, C, D).astype(bf)
    x2 = np.asarray(x2, dtype=np.float32).reshape(B, C, D).astype(bf)
    # chunk-major transposed layout: [b, dc, p, c] with d = dc*128 + p
    x1t = np.ascontiguousarray(
        x1.reshape(B, C, DC, 128).transpose(0, 2, 3, 1)
    )
    x2t = np.ascontiguousarray(
        x2.reshape(B, C, DC, 128).transpose(0, 2, 3, 1)
    )
    ident = np.eye(128, dtype=bf)

    in_maps = []
    for i in range(N_CORES):
        sl = slice(i * B_PER_CORE, (i + 1) * B_PER_CORE)
        in_maps.append(
            {
                "x1": np.ascontiguousarray(x1[sl].reshape(B_PER_CORE * C, D)),
                "x2": np.ascontiguousarray(x2[sl].reshape(B_PER_CORE * C, D)),
                "x1t": x1t[sl].reshape(B_PER_CORE * D, C),
                "x2t": x2t[sl].reshape(B_PER_CORE * D, C),
                "ident": ident,
            }
        )
    return in_maps


def kernel(x1: np.ndarray, x2: np.ndarray):
    from concourse.bass_utils import run_bass_kernel_spmd

    nc = _get_nc()
    in_maps = make_in_maps(x1, x2)

    res = None
    for attempt in range(3):
        try:
            res = run_bass_kernel_spmd(nc, in_maps, list(range(N_CORES))).results
            break
        except Exception:
            if attempt == 2:
                raise
    assert res is not None

    outA = np.empty((B, C, 64, 64), dtype=np.float32)
    outB = np.empty((B, C, 64, 64), dtype=np.float32)
    for i in range(N_CORES):
        sl = slice(i * B_PER_CORE, (i + 1) * B_PER_CORE)
        outA[sl] = res[i]["outA"].astype(np.float32).reshape(B_PER_CORE, C, 64, 64)
        outB[sl] = res[i]["outB"].astype(np.float32).reshape(B_PER_CORE, C, 64, 64)
    return outA, outB


# revision 9
# speedup vs baseline: 1.3961x; 1.0830x over previous
"""Mutual channel attention (sparse_attention) TRN2 Bass kernel — v2 (bf16).

Problem: x1, x2 of shape (16, 512, 64, 64) fp32.
  q = x1.reshape(B, C, D), k = x2.reshape(B, C, D), D = 4096, scale = 1/64
  S   = q @ k^T * scale                       [B, 512, 512]
  outA = softmax_rows(S) @ k                  -> (16, 512, 64, 64)
  outB = softmax_rows(S^T) @ q                -> (16, 512, 64, 64)

Key algebra: without max-subtraction (scores ~ N(0,1), safe in fp32),
P = exp(S*scale) serves BOTH directions; only the normalization sums
differ (row sums of P for A, column sums of P for B).

v2 vs v1 (285 us baseline):
- All matmuls in bf16 (1 cyc/row, rel err ~3e-3 vs 2e-2 budget). The
  host casts inputs to bf16 and also provides q^T/k^T in a chunked
  layout, so the per-d-chunk PE transposes (40% extra PE work in v1)
  disappear entirely. Outputs return as bf16 and are upcast on host.
- PE work per batch is just the three 512x512x4096 GEMMs + 16 P-block
  transposes: ~199k cycles -> ~166 us/core for 2 batches at 2.4 GHz.
- DMA per core: 33.6 MB in + 16.8 MB out = 50.4 MB (~141 us at 358
  GB/s), fully overlapped with compute.

Sharding: pure data parallel, 2 batches per core across 8 cores.

Per-core per-batch schedule:
  1. qT/kT piece loads (sync ring), k/q natural loads (scalar ring).
  2. Scores: per 128-d-chunk, 4 accumulating matmuls into resident
     S psum banks (stationary = qT slice, moving = kT chunk).
  3. exp via ACT with fused *1/64 scale and fused row-sum (dir A).
  4. PE-transpose P -> P_ec with fused column-sum on the ACT copy-out
     (dir B); DVE reciprocals.
  5. out_a = P_ec.T @ k, cc-outer with [128,4096] bf16 staging per
     c-chunk, normalization folded into the PSUM->SBUF copy
     (alternating DVE/ACT), one 1 MB store per chunk. Then
     out_b = P_ce.T @ q symmetrically.
"""

import numpy as np

B, C, D = 16, 512, 4096
N_CORES = 8
B_PER_CORE = B // N_CORES  # 2
CC = C // 128  # 4 c-chunks
DC = D // 128  # 32 d-chunks
NP = 4  # qT/kT pieces per batch
PCH = DC // NP  # 8 d-chunks per piece
NG = D // 512  # 8 d-groups of 512 in the out phase

_COMPILED = {}


def _build():
    import concourse.mybir as mybir
    from concourse import bacc, tile

    f32 = mybir.dt.float32
    bf16 = mybir.dt.bfloat16
    AF = mybir.ActivationFunctionType
    ROWS = B_PER_CORE * C  # 1024

    nc = bacc.Bacc(None, target_bir_lowering=False)
    x1 = nc.declare_dram_parameter("x1", [ROWS, D], bf16, isOutput=False)
    x2 = nc.declare_dram_parameter("x2", [ROWS, D], bf16, isOutput=False)
    # Transposed copies in piece-contiguous layout: row (b*NP + a)*128 + p
    # holds [di, c] flattened (PCH*C elems) with value q[b, c, (a*PCH+di)*128+p].
    # Each SBUF partition's piece data is one contiguous 8 KB run on both
    # sides -> full-size DMA descriptors (1 KB descriptors measured 55 GB/s
    # under queue contention vs ~360 GB/s for 8 KB ones).
    x1t = nc.declare_dram_parameter("x1t", [ROWS, D], bf16, isOutput=False)
    x2t = nc.declare_dram_parameter("x2t", [ROWS, D], bf16, isOutput=False)
    ident = nc.declare_dram_parameter("ident", [128, 128], bf16, isOutput=False)
    outA = nc.declare_dram_parameter("outA", [ROWS, D], bf16, isOutput=True)
    outB = nc.declare_dram_parameter("outB", [ROWS, D], bf16, isOutput=True)

    with tile.TileContext(nc) as tc:
        with (
            tc.tile_pool(name="const", bufs=1) as constp,
            tc.tile_pool(name="tp", bufs=1) as tp,
            tc.tile_pool(name="nat", bufs=1) as nat,
            tc.tile_pool(name="pp", bufs=1) as pp,
            tc.tile_pool(name="rp", bufs=2) as rp,
            tc.tile_pool(name="ost", bufs=3) as ost,
            tc.tile_pool(name="sps", bufs=1, space="PSUM") as sps,
            tc.tile_pool(name="ops", bufs=4, space="PSUM") as ops,
        ):
            idt = constp.tile([128, 128], bf16)
            nc.scalar.dma_start(idt[:], ident[:])

            for b in range(B_PER_CORE):
                r0 = b * C
                # ---- all loads on the sync ring, in consumption order:
                # qT/kT pieces (scores) interleaved with k chunks (out_a),
                # then q chunks (out_b). One ring = strict priority, no
                # packet-RR bandwidth steal from a second load queue.
                qT = [None] * NP
                kT = [None] * NP
                k = [None] * CC
                q = [None] * CC

                def t_rows(a, _b=b):
                    return slice((_b * NP + a) * 128, (_b * NP + a + 1) * 128)

                def n_rows(cc, _r0=r0):
                    return slice(_r0 + cc * 128, _r0 + (cc + 1) * 128)

                for a in range(NP):
                    qt = tp.tile([128, PCH * C], bf16, tag=f"qT{a}", name=f"qT{a}")
                    nc.sync.dma_start(qt[:], x1t[t_rows(a), :])
                    kt = tp.tile([128, PCH * C], bf16, tag=f"kT{a}", name=f"kT{a}")
                    nc.sync.dma_start(kt[:], x2t[t_rows(a), :])
                    qT[a] = qt
                    kT[a] = kt
                    if a >= 2:
                        for cc in (2 * a - 4, 2 * a - 3):
                            ktile = nat.tile(
                                [128, D], bf16, tag=f"k{cc}", name=f"k{cc}"
                            )
                            nc.sync.dma_start(ktile[:], x2[n_rows(cc), :])
                            k[cc] = ktile
                for cc in (2, 3):
                    ktile = nat.tile([128, D], bf16, tag=f"k{cc}", name=f"k{cc}")
                    nc.sync.dma_start(ktile[:], x2[n_rows(cc), :])
                    k[cc] = ktile
                for cc in range(CC):
                    qtile = nat.tile([128, D], bf16, tag=f"q{cc}", name=f"q{cc}")
                    nc.sync.dma_start(qtile[:], x1[n_rows(cc), :])
                    q[cc] = qtile

                # ---- scores: S_ce[cc] accumulates over 32 d-chunks ----
                s_ps = [
                    sps.tile([128, C], f32, tag=f"s{cc}", name=f"s{cc}")
                    for cc in range(CC)
                ]
                for dc in range(DC):
                    a, di = divmod(dc, PCH)
                    o = di * C
                    for cc in range(CC):
                        nc.tensor.matmul(
                            s_ps[cc][:],
                            qT[a][:, o + cc * 128 : o + (cc + 1) * 128],
                            kT[a][:, o : o + C],
                            start=(dc == 0),
                            stop=(dc == DC - 1),
                        )

                # ---- exp + row sums (direction A) ----
                p_ce = []
                rinv_a = []
                for cc in range(CC):
                    p = pp.tile([128, C], bf16, tag=f"pce{cc}", name=f"pce{cc}")
                    rs = rp.tile([128, 1], f32, tag=f"rsa{cc}", name=f"rsa{cc}")
                    nc.scalar.activation(
                        p[:], s_ps[cc][:], AF.Exp, scale=1.0 / 64.0, accum_out=rs[:]
                    )
                    ri = rp.tile([128, 1], f32, tag=f"ria{cc}", name=f"ria{cc}")
                    nc.vector.reciprocal(ri[:], rs[:])
                    p_ce.append(p)
                    rinv_a.append(ri)

                # ---- transpose P -> P_ec + column sums (direction B) ----
                stg = [
                    ops.tile([128, C], bf16, tag="st", name=f"pt{ec}")
                    for ec in range(CC)
                ]
                for cc in range(CC):
                    for ec in range(CC):
                        nc.tensor.transpose(
                            stg[ec][:, cc * 128 : (cc + 1) * 128],
                            p_ce[cc][:, ec * 128 : (ec + 1) * 128],
                            idt[:],
                        )
                p_ec = []
                rinv_b = []
                for ec in range(CC):
                    p = pp.tile([128, C], bf16, tag=f"pec{ec}", name=f"pec{ec}")
                    rs = rp.tile([128, 1], f32, tag=f"rsb{ec}", name=f"rsb{ec}")
                    nc.scalar.activation(p[:], stg[ec][:], AF.Copy, accum_out=rs[:])
                    ri = rp.tile([128, 1], f32, tag=f"rib{ec}", name=f"rib{ec}")
                    nc.vector.reciprocal(ri[:], rs[:])
                    p_ec.append(p)
                    rinv_b.append(ri)

                # ---- out_a = (P_ec.T @ k) * rinv_a, cc-outer, wide staging ----
                for cc in range(CC):
                    csl = slice(cc * 128, (cc + 1) * 128)
                    oa = ost.tile([128, D], bf16, tag="ost", name=f"oa{cc}")
                    for g in range(NG):
                        dsl = slice(g * 512, (g + 1) * 512)
                        o_ps = ops.tile([128, 512], f32, tag="st", name="oa_ps")
                        for ec in range(CC):
                            nc.tensor.matmul(
                                o_ps[:],
                                p_ec[ec][:, csl],
                                k[ec][:, dsl],
                                start=(ec == 0),
                                stop=(ec == CC - 1),
                            )
                        if g % 2 == 0:
                            nc.vector.tensor_scalar_mul(
                                oa[:, dsl], o_ps[:], rinv_a[cc][:]
                            )
                        else:
                            nc.scalar.activation(
                                oa[:, dsl], o_ps[:], AF.Copy, scale=rinv_a[cc][:]
                            )
                    nc.scalar.dma_start(
                        outA[r0 + cc * 128 : r0 + (cc + 1) * 128, :], oa[:]
                    )

                # ---- out_b = (P_ce.T @ q) * rinv_b, ec-outer, wide staging ----
                for ec in range(CC):
                    esl = slice(ec * 128, (ec + 1) * 128)
                    ob = ost.tile([128, D], bf16, tag="ost", name=f"ob{ec}")
                    for g in range(NG):
                        dsl = slice(g * 512, (g + 1) * 512)
                        o_ps = ops.tile([128, 512], f32, tag="st", name="ob_ps")
                        for cc in range(CC):
                            nc.tensor.matmul(
                                o_ps[:],
                                p_ce[cc][:, esl],
                                q[cc][:, dsl],
                                start=(cc == 0),
                                stop=(cc == CC - 1),
                            )
                        if g % 2 == 0:
                            nc.vector.tensor_scalar_mul(
                                ob[:, dsl], o_ps[:], rinv_b[ec][:]
                            )
                        else:
                            nc.scalar.activation(
                                ob[:, dsl], o_ps[:], AF.Copy, scale=rinv_b[ec][:]
                            )
                    nc.scalar.dma_start(
                        outB[r0 + ec * 128 : r0 + (ec + 1) * 128, :], ob[:]
                    )

    nc.finalize()
    return nc


def _get_nc():
    if "nc" not in _COMPILED:
        _COMPILED["nc"] = _build()
    return _COMPILED["nc"]


def make_in_maps(x1: np.ndarray, x2: np.ndarray):
    import ml_dtypes

    bf = ml_dtypes.bfloat16
    x1 = np.asarray(x1, dtype=np.float32).reshape(B, C, D).astype(bf)
    x2 = np.asarray(x2, dtype=np.float32).reshape(B, C, D).astype(bf)
    # piece-contiguous transposed layout: row (b*NP + a)*128 + p holds
    # [di, c] flattened, value = x[b, c, (a*PCH+di)*128 + p]
    x1t = np.ascontiguousarray(
        x1.reshape(B, C, NP, PCH, 128).transpose(0, 2, 4, 3, 1)
    )
    x2t = np.ascontiguousarray(
        x2.reshape(B, C, NP, PCH, 128).transpose(0, 2, 4, 3, 1)
    )
    ident = np.eye(128, dtype=bf)

    in_maps = []
    for i in range(N_CORES):
        sl = slice(i * B_PER_CORE, (i + 1) * B_PER_CORE)
        in_maps.append(
            {
                "x1": np.ascontiguousarray(x1[sl].reshape(B_PER_CORE * C, D)),
                "x2": np.ascontiguousarray(x2[sl].reshape(B_PER_CORE * C, D)),
                "x1t": x1t[sl].reshape(B_PER_CORE * C, D),
                "x2t": x2t[sl].reshape(B_PER_CORE * C, D),
                "ident": ident,
            }
        )
    return in_maps


def kernel(x1: np.ndarray, x2: np.ndarray):
    from concourse.bass_utils import run_bass_kernel_spmd

    nc = _get_nc()
    in_maps = make_in_maps(x1, x2)

    res = None
    for attempt in range(3):
        try:
            res = run_bass_kernel_spmd(nc, in_maps, list(range(N_CORES))).results
            break
        except Exception:
            if attempt == 2:
                raise
    assert res is not None

    outA = np.empty((B, C, 64, 64), dtype=np.float32)
    outB = np.empty((B, C, 64, 64), dtype=np.float32)
    for i in range(N_CORES):
        sl = slice(i * B_PER_CORE, (i + 1) * B_PER_CORE)
        outA[sl] = res[i]["outA"].astype(np.float32).reshape(B_PER_CORE, C, 64, 64)
        outB[sl] = res[i]["outB"].astype(np.float32).reshape(B_PER_CORE, C, 64, 64)
    return outA, outB
